# revision 1
# baseline (speedup 1.0000x reference)
"""Trainium2 Bass kernel for nn_CalibrationModelObsGridGeometry.

Pipeline: gather -> gaussian pyramid (75-tap, 11 sigmas) -> BatchNorm ->
3-layer 3x3 CNN -> scatter.  Sharded data-parallel over 24 gathered swaths
across 8 NeuronCores (3 swaths/core).

Device phase A: difference-of-gaussian Toeplitz-band matmuls produce the 12
unique cal_input channels (channels 11..20 of the reference duplicate 0..9).
Host: BN batch stats from cal, folded into conv1 weights (pad-with-mean).
Device phase B: 3x3 convs as 9 accumulating matmuls with flat free-dim
offsets; 4 h-quarters processed in parallel via block-diagonal weights across
partition groups; ACT applies bias+ReLU on PSUM eviction.
Host: + fs_sel + const, scatter-add, mask.
"""

import numpy as np

# ---------------------------------------------------------------- constants
B, P, H, W = 4, 8, 1200, 52
M_SEL, HI = 24, 1100
SIZE = 75
HALF = SIZE // 2  # 37
SIGS = tuple(8 * (i + 1) for i in range(10))
NS = (0.31446309894037083, 0.3886609494201447)
BN_EPS = 1e-5
HID = 32
NCORES = 8
SW = 3                      # swaths per core
NWIN = 21                   # toeplitz windows per swath (54 out rows each)
WJ = 54                     # out rows per window
HREC = NWIN * WJ            # 1134 recorded rows (>=1100; tail garbage)
NQ = 4                      # h-quarters (partition groups)
QROWS = HI // NQ            # 275
NT = 5                      # processing tiles per swath
R = QROWS // NT             # 55 out rows per tile per quarter
W2 = 54                     # padded width
CAL_ROWS = R + 6            # 61 stored cal rows per tile
H1_ROWS = R + 4             # 59
H2_ROWS = R + 2             # 57
CAL_F = CAL_ROWS * W2       # 3294
H1_F = H1_ROWS * W2         # 3186
H2_F = H2_ROWS * W2         # 3078
O_F = R * W2                # 2970
CAL_SZ = CAL_F + 2          # +1 lead, +1 tail guard
H1_SZ = H1_F + 2
H2_SZ = H2_F + 2
CHUNK = 486                 # <=512 fp32 psum-bank limit
NST = SW * NT               # 15 processing tiles per core

EMULATE = False             # numpy-emulate the device kernels (debug)


def _gauss1d(size, sig):
    x = np.arange(size, dtype=np.float32) - (size - 1) / 2.0
    g = np.exp(-(x ** 2) / (2.0 * sig ** 2))
    return (g / g.sum()).astype(np.float32)


def _bands():
    """12 cal channels as 75-tap bands: D0..D9, A(=G9 on fy), B(=G9 on fs)."""
    g = np.stack([_gauss1d(SIZE, s) for s in SIGS])  # [10, 75]
    bands = np.zeros((12, SIZE), np.float32)
    bands[0] = -g[0]
    bands[0, HALF] += 1.0
    for i in range(1, 10):
        bands[i] = g[i - 1] - g[i]
    bands[10] = g[9]
    bands[11] = g[9]
    return bands


def _toeplitz():
    """lhsT [12,128,54]: per-channel Toeplitz bands (M=54 out rows/window)."""
    bands = _bands()
    toep = np.zeros((12, 128, WJ), np.float32)
    for ch in range(12):
        for j in range(WJ):
            toep[ch, j:j + SIZE, j] = bands[ch]
    return toep


def _chunks(total):
    out = []
    off = 0
    while off < total:
        sz = min(CHUNK, total - off)
        out.append((off, sz))
        off += sz
    return out


# ---------------------------------------------------------------- device build
_CACHE = {}


def _get_bass():
    import tile_patch_inline  # noqa: F401  (placeholder; patch applied below)


def _apply_tile_patch():
    import concourse.tile as tile
    from concourse import mybir
    from concourse.vector_clock import ScopedClock

    def _patched(self, tick_clock, wait_clock):
        nc = self.nc
        drain_inst = nc.sync.drain()
        wait_clock.add_sem_waits(
            drain_inst.ins, ScopedClock({None: tick_clock.global_clock})
        )
        si = drain_inst.ins.sync_info
        if si is not None and si.on_wait and len(si.on_wait) > 1:
            extra = list(si.on_wait[1:])
            del si.on_wait[1:]
            for w in extra:
                d2 = nc.sync.drain()
                si2 = d2.ins.sync_info
                if si2 is None:
                    d2.ins.sync_info = mybir.SyncInfo(on_wait=[w], on_update=[])
                else:
                    si2.on_wait.append(w)
        nc.all_engine_barrier()
        popped = nc._tile_sem_poison_stack.pop()
        assert popped is self._sem_poison
        nc.clear_and_free_semaphores(list(self.sems.allocated().values()))
        nc.all_engine_barrier()

    tile.TileContext._drain_and_barrier = _patched


_WSPLIT_N = [0]


def _split_waits(nc):
    """This walrus build accepts only one sync-wait per instruction: hoist
    extra waits onto same-engine NoOps placed just before the instruction."""
    from concourse import mybir
    for f in nc.m.functions:
        for bb in f.blocks:
            new_list = []
            for ins in bb.instructions:
                si = getattr(ins, "sync_info", None)
                if si is not None and si.on_wait and len(si.on_wait) > 1:
                    extra = list(si.on_wait[:-1])
                    del si.on_wait[:-1]
                    for w in extra:
                        _WSPLIT_N[0] += 1
                        nop = mybir.InstDrain(
                            name=f"WSPLIT-{_WSPLIT_N[0]}",
                            engine=ins.engine,
                            sync_info=mybir.SyncInfo(on_wait=[w], on_update=[]),
                            bass_is_fusable=False,
                        )
                        new_list.append(nop)
                new_list.append(ins)
            bb.instructions[:] = new_list


def _build_phase_a():
    import concourse.bass as bass
    import concourse.tile as tile
    from concourse import mybir

    f32 = mybir.dt.float32
    nc = bass.Bass("TRN2")
    fyw = nc.dram_tensor("fyw", [NWIN, 128, SW * W], f32, kind="ExternalInput")
    fsw = nc.dram_tensor("fsw", [NWIN, 128, SW * W], f32, kind="ExternalInput")
    toep = nc.dram_tensor("toep", [12, 128, WJ], f32, kind="ExternalInput")
    NF = SW * W  # 156
    # staging-order layout: [wt, ch, j, i, s*52+w]; host decodes h=54*(3wt+i)+j
    cal = nc.dram_tensor("cal", [NWIN // 3, 12, WJ, 3, NF], f32,
                         kind="ExternalOutput")

    with tile.TileContext(nc) as tc:
        with (
            tc.tile_pool(name="singles", bufs=1) as singles,
            tc.tile_pool(name="stage", bufs=3) as stage,
            tc.tile_pool(name="psum", bufs=4, space="PSUM") as psum,
        ):
            fyw_s = singles.tile([128, NWIN, NF], f32)
            nc.sync.dma_start(out=fyw_s[:], in_=fyw[:].rearrange("w k f -> k w f"))
            fsw_s = singles.tile([128, NWIN, NF], f32)
            nc.sync.dma_start(out=fsw_s[:], in_=fsw[:].rearrange("w k f -> k w f"))
            toep_t = []
            for ch in range(12):
                tt = singles.tile([128, WJ], f32, tag=f"toep{ch}")
                nc.sync.dma_start(out=tt[:], in_=toep[ch])
                toep_t.append(tt)

            for wt in range(NWIN // 3):
                for ch in range(12):
                    src_s = fsw_s if ch == 11 else fyw_s
                    st = stage.tile([WJ, 3, NF], f32, tag="stA")
                    for i in range(3):
                        w = wt * 3 + i
                        ps = psum.tile([WJ, NF], f32, tag="psA")
                        nc.tensor.matmul(
                            ps[:], lhsT=toep_t[ch][:],
                            rhs=src_s[:, w, :], start=True, stop=True,
                        )
                        nc.scalar.copy(st[:, i, :], ps[:])
                    nc.sync.dma_start(out=cal[wt, ch], in_=st[:])
    _split_waits(nc)
    return nc


def _build_phase_b():
    import concourse.bass as bass
    import concourse.tile as tile
    from concourse import mybir

    f32 = mybir.dt.float32
    nc = bass.Bass("TRN2")
    calp = nc.dram_tensor("calp", [48, NST, CAL_SZ], f32, kind="ExternalInput")
    l1 = nc.dram_tensor("l1", [9, 48, 128], f32, kind="ExternalInput")
    l2 = nc.dram_tensor("l2", [9, 128, 128], f32, kind="ExternalInput")
    l3 = nc.dram_tensor("l3", [9, 128, 4], f32, kind="ExternalInput")
    b1 = nc.dram_tensor("b1t", [128, 1], f32, kind="ExternalInput")
    b2 = nc.dram_tensor("b2t", [128, 1], f32, kind="ExternalInput")
    b3 = nc.dram_tensor("b3t", [4, 1], f32, kind="ExternalInput")
    o = nc.dram_tensor("o", [NST, 4, O_F], f32, kind="ExternalOutput")

    Relu = mybir.ActivationFunctionType.Relu
    Ident = mybir.ActivationFunctionType.Identity

    with tile.TileContext(nc) as tc:
        with (
            tc.tile_pool(name="wts", bufs=1) as wts,
            tc.tile_pool(name="io", bufs=3) as io,
            tc.tile_pool(name="acts", bufs=3) as acts,
            tc.tile_pool(name="psum", bufs=6, space="PSUM") as psum,
            tc.tile_pool(name="psum3", bufs=2, space="PSUM") as psum3,
        ):
            w1s = wts.tile([48, 9, 128], f32)
            nc.sync.dma_start(out=w1s[:], in_=l1[:].rearrange("t k m -> k t m"))
            w2s = wts.tile([128, 9, 128], f32)
            nc.sync.dma_start(out=w2s[:], in_=l2[:].rearrange("t k m -> k t m"))
            w3s = wts.tile([128, 9, 4], f32)
            nc.sync.dma_start(out=w3s[:], in_=l3[:].rearrange("t k m -> k t m"))
            b1s = wts.tile([128, 1], f32)
            nc.sync.dma_start(out=b1s[:], in_=b1[:])
            b2s = wts.tile([128, 1], f32)
            nc.sync.dma_start(out=b2s[:], in_=b2[:])
            b3s = wts.tile([4, 1], f32)
            nc.sync.dma_start(out=b3s[:], in_=b3[:])

            for st_i in range(NST):
                t_i = st_i % NT
                calt = io.tile([48, CAL_SZ], f32, tag="cal")
                nc.sync.dma_start(out=calt[:], in_=calp[:, st_i, :])

                h1 = acts.tile([128, H1_SZ], f32, tag="h1")
                h2 = acts.tile([128, H2_SZ], f32, tag="h2")
                ot = io.tile([4, O_F], f32, tag="ot")

                # ---- conv1: cal[48] -> h1[128], ReLU(. + b1)
                for off, sz in _chunks(H1_F):
                    ps = psum.tile([128, CHUNK], f32, tag="ps")
                    for t9 in range(9):
                        dy, dx = t9 // 3 - 1, t9 % 3 - 1
                        base = off + W2 * (1 + dy) + dx + 1
                        nc.tensor.matmul(
                            ps[:, :sz], lhsT=w1s[:, t9, :],
                            rhs=calt[:, base:base + sz],
                            start=(t9 == 0), stop=(t9 == 8),
                        )
                    nc.scalar.activation(
                        out=h1[:, 1 + off:1 + off + sz], in_=ps[:, :sz],
                        func=Relu, bias=b1s[:, 0:1], scale=1.0,
                    )
                # zero the width-pad columns of h1
                h1v = h1[:, 1:1 + H1_F].rearrange("p (r c) -> p r c", c=W2)
                nc.vector.memset(h1v[:, :, 0:1], 0.0)
                nc.vector.memset(h1v[:, :, W2 - 1:W2], 0.0)
                if t_i == 0:      # swath top: zero rows of quarter 0
                    nc.vector.memset(h1[0:32, 1:1 + 2 * W2], 0.0)
                if t_i == NT - 1:  # swath bottom: zero rows of quarter 3
                    nc.vector.memset(
                        h1[96:128, 1 + (H1_ROWS - 2) * W2:1 + H1_F], 0.0)

                # ---- conv2: h1[128] -> h2[128], ReLU(. + b2)
                for off, sz in _chunks(H2_F):
                    ps = psum.tile([128, CHUNK], f32, tag="ps")
                    for t9 in range(9):
                        dy, dx = t9 // 3 - 1, t9 % 3 - 1
                        base = off + W2 * (1 + dy) + dx + 1
                        nc.tensor.matmul(
                            ps[:, :sz], lhsT=w2s[:, t9, :],
                            rhs=h1[:, base:base + sz],
                            start=(t9 == 0), stop=(t9 == 8),
                        )
                    nc.scalar.activation(
                        out=h2[:, 1 + off:1 + off + sz], in_=ps[:, :sz],
                        func=Relu, bias=b2s[:, 0:1], scale=1.0,
                    )
                h2v = h2[:, 1:1 + H2_F].rearrange("p (r c) -> p r c", c=W2)
                nc.vector.memset(h2v[:, :, 0:1], 0.0)
                nc.vector.memset(h2v[:, :, W2 - 1:W2], 0.0)
                if t_i == 0:
                    nc.vector.memset(h2[0:32, 1:1 + W2], 0.0)
                if t_i == NT - 1:
                    nc.vector.memset(
                        h2[96:128, 1 + (H2_ROWS - 1) * W2:1 + H2_F], 0.0)

                # ---- conv3: h2[128] -> o[4], Identity(. + b3')
                for off, sz in _chunks(O_F):
                    ps = psum3.tile([4, CHUNK], f32, tag="ps3")
                    for t9 in range(9):
                        dy, dx = t9 // 3 - 1, t9 % 3 - 1
                        base = off + W2 * (1 + dy) + dx + 1
                        nc.tensor.matmul(
                            ps[:, :sz], lhsT=w3s[:, t9, :],
                            rhs=h2[:, base:base + sz],
                            start=(t9 == 0), stop=(t9 == 8),
                        )
                    nc.scalar.activation(
                        out=ot[:, off:off + sz], in_=ps[:, :sz],
                        func=Ident, bias=b3s[:, 0:1], scale=1.0,
                    )
                nc.sync.dma_start(out=o[st_i], in_=ot[:])
    _split_waits(nc)
    return nc


# ---------------------------------------------------------------- emulation
def _emulate_a(in_map):
    toep = in_map["toep"]
    fyw, fsw = in_map["fyw"], in_map["fsw"]
    NF = SW * W
    cal = np.zeros((NWIN // 3, 12, WJ, 3, NF), np.float32)
    for wt in range(NWIN // 3):
        for i in range(3):
            w = wt * 3 + i
            for ch in range(12):
                src = fsw if ch == 11 else fyw
                cal[wt, ch, :, i, :] = toep[ch].T @ src[w]
    return {"cal": cal}


def _decode_cal(cal2):
    """[7,12,54,3,156] -> [12, SW, 1100, 52]; h = 54*(3*wt+i)+j."""
    a = cal2.reshape(NWIN // 3, 12, WJ, 3, SW, W)
    a = a.transpose(1, 4, 0, 3, 2, 5).reshape(12, SW, HREC, W)
    return a[:, :, :HI, :]


def _emulate_b(in_map):
    calp, l1, l2, l3 = in_map["calp"], in_map["l1"], in_map["l2"], in_map["l3"]
    b1t, b2t, b3t = in_map["b1t"], in_map["b2t"], in_map["b3t"]
    o = np.zeros((NST, 4, O_F), np.float32)
    for st_i in range(NST):
        t_i = st_i % NT
        calt = calp[:, st_i, :]
        h1 = np.zeros((128, H1_SZ), np.float32)
        h2 = np.zeros((128, H2_SZ), np.float32)
        acc = np.zeros((128, H1_F), np.float32)
        for t9 in range(9):
            dy, dx = t9 // 3 - 1, t9 % 3 - 1
            base = W2 * (1 + dy) + dx + 1
            acc += l1[t9].T @ calt[:, base:base + H1_F]
        h1[:, 1:1 + H1_F] = np.maximum(acc + b1t, 0.0)
        h1v = h1[:, 1:1 + H1_F].reshape(128, H1_ROWS, W2)
        h1v[:, :, 0] = 0.0
        h1v[:, :, W2 - 1] = 0.0
        if t_i == 0:
            h1[0:32, 1:1 + 2 * W2] = 0.0
        if t_i == NT - 1:
            h1[96:128, 1 + (H1_ROWS - 2) * W2:1 + H1_F] = 0.0
        acc = np.zeros((128, H2_F), np.float32)
        for t9 in range(9):
            dy, dx = t9 // 3 - 1, t9 % 3 - 1
            base = W2 * (1 + dy) + dx + 1
            acc += l2[t9].T @ h1[:, base:base + H2_F]
        h2[:, 1:1 + H2_F] = np.maximum(acc + b2t, 0.0)
        h2v = h2[:, 1:1 + H2_F].reshape(128, H2_ROWS, W2)
        h2v[:, :, 0] = 0.0
        h2v[:, :, W2 - 1] = 0.0
        if t_i == 0:
            h2[0:32, 1:1 + W2] = 0.0
        if t_i == NT - 1:
            h2[96:128, 1 + (H2_ROWS - 1) * W2:1 + H2_F] = 0.0
        acc = np.zeros((4, O_F), np.float32)
        for t9 in range(9):
            dy, dx = t9 // 3 - 1, t9 % 3 - 1
            base = W2 * (1 + dy) + dx + 1
            acc += l3[t9].T @ h2[:, base:base + O_F]
        o[st_i] = acc + b3t
    return {"o": o}


def _run(phase, in_maps):
    """Run phase ('a'|'b') on 8 cores; returns list of output dicts."""
    if EMULATE:
        em = _emulate_a if phase == "a" else _emulate_b
        return [em(m) for m in in_maps]
    if "nc_" + phase not in _CACHE:
        _apply_tile_patch()
        _CACHE["nc_" + phase] = (
            _build_phase_a() if phase == "a" else _build_phase_b()
        )
    from concourse.bass_utils import run_bass_kernel_spmd
    import time as _time
    t0 = _time.time()
    res = run_bass_kernel_spmd(
        _CACHE["nc_" + phase], in_maps, core_ids=list(range(NCORES)),
    )
    _CACHE.setdefault("wall_ns", {})[phase] = int((_time.time() - t0) * 1e9)
    if res.exec_time_ns is not None:
        _CACHE.setdefault("exec_ns", {})[phase] = res.exec_time_ns
    return res.results


# ---------------------------------------------------------------- main entry
def kernel(sv_uncal, sv_bg, kernel, w1, b1, w2, b2, w3, b3, msk_idx, row_idx):
    sv_uncal = np.asarray(sv_uncal, np.float32)
    sv_bg = np.asarray(sv_bg, np.float32)
    w1 = np.asarray(w1, np.float32)
    b1 = np.asarray(b1, np.float32)
    w2 = np.asarray(w2, np.float32)
    b2 = np.asarray(b2, np.float32)
    w3 = np.asarray(w3, np.float32)
    b3 = np.asarray(b3, np.float32)
    msk_idx = np.asarray(msk_idx)
    row_idx = np.asarray(row_idx)

    # ---- host gather + replicate pad + windowing
    fy = sv_uncal.reshape(B * P, H, W)[msk_idx][:, row_idx]   # [24, 1100, 52]
    fs = sv_bg.reshape(B * P, H, W)[msk_idx][:, row_idx]
    fyp = np.pad(fy, ((0, 0), (HALF, HALF), (0, 0)), mode="edge")
    fsp = np.pad(fs, ((0, 0), (HALF, HALF), (0, 0)), mode="edge")
    need = WJ * (NWIN - 1) + 128                              # 1208
    fyp = np.pad(fyp, ((0, 0), (0, need - fyp.shape[1]), (0, 0)))
    fsp = np.pad(fsp, ((0, 0), (0, need - fsp.shape[1]), (0, 0)))
    widx = WJ * np.arange(NWIN)[:, None] + np.arange(128)[None]
    fyw_all = fyp[:, widx, :]                                  # [24, 21, 128, 52]
    fsw_all = fsp[:, widx, :]
    toep = _toeplitz()

    in_maps_a = []
    for c in range(NCORES):
        sl = slice(SW * c, SW * c + SW)
        fyw = np.ascontiguousarray(
            fyw_all[sl].transpose(1, 2, 0, 3).reshape(NWIN, 128, SW * W))
        fsw = np.ascontiguousarray(
            fsw_all[sl].transpose(1, 2, 0, 3).reshape(NWIN, 128, SW * W))
        in_maps_a.append(dict(fyw=fyw, fsw=fsw, toep=toep))

    res_a = _run("a", in_maps_a)
    cal_all = np.concatenate(
        [_decode_cal(r["cal"]) for r in res_a], axis=1)       # [12, 24, 1100, 52]

    # ---- BN stats + weight folding (host)
    m64 = cal_all.astype(np.float64).mean(axis=(1, 2, 3))
    v64 = cal_all.astype(np.float64).var(axis=(1, 2, 3))
    r64 = 1.0 / np.sqrt(v64 + BN_EPS)
    mch = m64.astype(np.float32)
    rch = r64.astype(np.float32)

    w1f = np.concatenate(
        [w1[:, 0:10] + w1[:, 11:21], w1[:, 10:11], w1[:, 21:22]], axis=1)
    w1e = w1f * rch[None, :, None, None]                      # [32, 12, 3, 3]
    b1e = b1 - np.einsum("ocyx,c->o", w1f, rch * mch)

    l1 = np.zeros((9, 48, 128), np.float32)
    l2 = np.zeros((9, 128, 128), np.float32)
    l3 = np.zeros((9, 128, 4), np.float32)
    for t9 in range(9):
        dy, dx = t9 // 3, t9 % 3
        for q in range(NQ):
            l1[t9, 12 * q:12 * q + 12, 32 * q:32 * q + 32] = w1e[:, :, dy, dx].T
            l2[t9, 32 * q:32 * q + 32, 32 * q:32 * q + 32] = w2[:, :, dy, dx].T
            l3[t9, 32 * q:32 * q + 32, q] = w3[0, :, dy, dx]
    b1t = np.tile(b1e, NQ)[:, None].astype(np.float32)
    b2t = np.tile(b2, NQ)[:, None].astype(np.float32)
    b3t = np.full((4, 1), b3[0] + np.float32(NS[0] / NS[1]), np.float32)

    # ---- padded quartered conv input
    in_maps_b = []
    rr = np.arange(CAL_ROWS)
    q_i = np.arange(NQ)
    t_j = np.arange(NT)
    grow = (QROWS * q_i[:, None, None] + R * t_j[None, :, None]
            + rr[None, None, :] - 3)                          # [4, 5, 61]
    for c in range(NCORES):
        calc = cal_all[:, SW * c:SW * c + SW]                 # [12, 3, 1100, 52]
        pad = np.empty((12, SW, HI + 6, W2), np.float32)
        pad[:] = mch[:, None, None, None]
        pad[:, :, 3:3 + HI, 1:1 + W] = calc
        g = pad[:, :, grow + 3, :]                            # [12, 3, 4, 5, 61, 54]
        calp = np.zeros((48, NST, CAL_SZ), np.float32)
        calp[:, :, 1:1 + CAL_F] = (
            g.transpose(2, 0, 1, 3, 4, 5)                     # [4,12,3,5,61,54]
            .reshape(48, SW, NT, CAL_F)
            .reshape(48, NST, CAL_F))
        in_maps_b.append(dict(calp=calp, l1=l1, l2=l2, l3=l3,
                              b1t=b1t, b2t=b2t, b3t=b3t))

    res_b = _run("b", in_maps_b)

    # ---- assemble + fs + scatter (host)
    outs = []
    for c in range(NCORES):
        oo = res_b[c]["o"].reshape(SW, NT, 4, R, W2)[:, :, :, :, 1:1 + W]
        outs.append(oo.transpose(0, 2, 1, 3, 4).reshape(SW, HI, W))
    o_dev = np.concatenate(outs, axis=0)                      # [24, 1100, 52]
    out = o_dev + fs

    out_cal = np.zeros((B * P, HI, W), np.float32)
    np.add.at(out_cal, msk_idx, out)
    cnt = np.zeros((B * P,), np.float32)
    np.add.at(cnt, msk_idx, 1.0)
    out_msk = np.broadcast_to(
        (cnt > 0)[:, None, None], (B * P, HI, W)).copy()
    return (out_cal.reshape(B, P, HI, W),
            out_msk.reshape(B, P, HI, W))

